# revision 15
# baseline (speedup 1.0000x reference)
"""Trainium2 Bass kernel for nn_Decoder_33088428049154.

Sharding: 8 cores; core c handles batch b=c//2 and T-half th=c%2.
Each core runs the full 6-layer decoder for its batch element (replicated
across the pair) and the joint network for its (64 t) x (64 u) x 4096 slice.

Host-side prep (inside kernel()): embedding gather + positional encoding,
causal-mask -> additive bias, LN gain/bias folded into the following weight
matrices, 1/sqrt(dh) folded into Wq, memory slice transposes, weight casts.

Device: decoder matmuls in bf16 (FWL weight loads, half DMA) with the
residual stream / layernorm stats / softmax kept in fp32; joint network in
float32r (full PE rate at N=512, ~1e-4 relative error). Joint z built
feature-major so tanh output directly feeds lhsT of the
(4096 rows x 512 K x 4096 N) output matmul; its bias is fused into the
PSUM-evacuation vector add via a host-replicated b_out.
"""

import math

import numpy as np

B, T, U = 4, 128, 64
D, H, DFF, J, ODIM, L = 512, 4, 2048, 512, 4096, 6
DH = D // H  # 128
KD = D // 128  # 4
KJ = J // 128  # 4
MD = DFF // 128  # 16
NOUT = ODIM // 512  # 8
TS = T // 2  # 64 timesteps per core
NCORES = 8
EPS = 1e-12

_CACHE = {}


def _build():
    import concourse.mybir as mybir
    from concourse import bacc, tile

    f32 = mybir.dt.float32
    f32r = mybir.dt.float32r
    bf16 = mybir.dt.bfloat16
    AF = mybir.ActivationFunctionType
    ALU = mybir.AluOpType
    AX = mybir.AxisListType

    nc = bacc.Bacc("TRN2", target_bir_lowering=False, debug=False, num_devices=8)

    # ---- DRAM I/O ----
    x0_d = nc.dram_tensor("x0", [U, D], f32, kind="ExternalInput")
    memT_d = nc.dram_tensor("memT", [D, TS], f32r, kind="ExternalInput")
    mask_d = nc.dram_tensor("maskb", [U, H * U], f32, kind="ExternalInput")
    identb_d = nc.dram_tensor("identb", [128, 128], bf16, kind="ExternalInput")
    identr_d = nc.dram_tensor("identr", [128, 128], f32r, kind="ExternalInput")
    onesb_d = nc.dram_tensor("onesb", [1, 128], bf16, kind="ExternalInput")
    onesr_d = nc.dram_tensor("onesr", [1, 128], f32r, kind="ExternalInput")
    wq_d = nc.dram_tensor("wq", [L, D, D], bf16, kind="ExternalInput")
    wk_d = nc.dram_tensor("wk", [L, D, D], bf16, kind="ExternalInput")
    wv_d = nc.dram_tensor("wv", [L, D, D], bf16, kind="ExternalInput")
    wo_d = nc.dram_tensor("wo", [L, D, D], bf16, kind="ExternalInput")
    w1_d = nc.dram_tensor("w1", [L, D, DFF], bf16, kind="ExternalInput")
    w2_d = nc.dram_tensor("w2", [L, DFF, D], bf16, kind="ExternalInput")
    bq_d = nc.dram_tensor("bq", [L, 128, H], f32, kind="ExternalInput")
    bk_d = nc.dram_tensor("bk", [L, 128, H], f32, kind="ExternalInput")
    bv_d = nc.dram_tensor("bv", [L, D], bf16, kind="ExternalInput")
    bo_d = nc.dram_tensor("bo", [L, D], bf16, kind="ExternalInput")
    b1_d = nc.dram_tensor("b1", [L, 128, MD], f32, kind="ExternalInput")
    b2_d = nc.dram_tensor("b2", [L, D], bf16, kind="ExternalInput")
    wenc_d = nc.dram_tensor("wenc", [D, J], f32r, kind="ExternalInput")
    benc_d = nc.dram_tensor("benc", [128, KJ], f32, kind="ExternalInput")
    wdec_d = nc.dram_tensor("wdec", [D, J], f32r, kind="ExternalInput")
    bdec_d = nc.dram_tensor("bdec", [128, KJ], f32, kind="ExternalInput")
    wout_d = nc.dram_tensor("wout", [J, ODIM], f32r, kind="ExternalInput")
    boutr_d = nc.dram_tensor("boutr", [128, ODIM], f32, kind="ExternalInput")
    out_d = nc.dram_tensor("out", [TS * U, ODIM], f32, kind="ExternalOutput")

    with tile.TileContext(nc) as tc:
        with (
            tc.tile_pool(name="const", bufs=1) as cpool,
            tc.tile_pool(name="bias", bufs=1) as bpool,
            tc.tile_pool(name="persist", bufs=1) as ppool,
            tc.tile_pool(name="sm", bufs=3) as sm,
            tc.tile_pool(name="act", bufs=2) as ap_,
            tc.tile_pool(name="jw", bufs=1) as jw,
        ):
            identb = cpool.tile([128, 128], bf16, tag="identb")
            identr = cpool.tile([128, 128], f32r, tag="identr")
            onesb = cpool.tile([1, 128], bf16, tag="onesb")
            onesr = cpool.tile([1, 128], f32r, tag="onesr")

            benc_sb = bpool.tile([128, KJ], f32, tag="benc")
            nc.gpsimd.dma_start(benc_sb[:], benc_d.ap())
            bdec_sb = bpool.tile([128, KJ], f32, tag="bdec")
            nc.gpsimd.dma_start(bdec_sb[:], bdec_d.ap())

            x0_sb = ppool.tile([U, D], f32, tag="x0")
            nc.gpsimd.dma_start(x0_sb[:], x0_d.ap())
            nc.gpsimd.dma_start(identb[:], identb_d.ap())
            nc.gpsimd.dma_start(onesb[:], onesb_d.ap())
            nc.gpsimd.dma_start(onesr[:], onesr_d.ap())
            mask_sb = ppool.tile([U, H * U], f32, tag="mask")
            nc.gpsimd.dma_start(mask_sb[:], mask_d.ap())
            memT_sb = ppool.tile([128, KD, TS], f32r, tag="memT")
            wenc_sb = ppool.tile([128, KD, J], f32r, tag="wenc")
            wdec_sb = ppool.tile([128, KD, J], f32r, tag="wdec")
            zencT = ppool.tile([128, KJ, TS], f32, tag="zencT")
            zdecT = ppool.tile([128, KJ, TS], f32, tag="zdecT")

            def layernorm(xin, out_dt, tag, s_pre=None):
                """Return xn = (xin - mean) / sqrt(var + eps), token-major.

                var = E[x^2] - mean^2 so the Square/accum (ACT) runs in
                parallel with the sum reduce (DVE); s_pre is a precomputed
                row-sum (from a fused residual-add accum_out) if available.
                """
                if s_pre is None:
                    s = sm.tile([U, 1], f32, tag=tag + "_s")
                    nc.vector.tensor_reduce(s[:], xin, axis=AX.X, op=ALU.add)
                else:
                    s = s_pre
                sq = ap_.tile([U, D], f32, tag="lnsq")
                ssq = sm.tile([U, 1], f32, tag=tag + "_ssq")
                nc.scalar.activation(sq[:], xin, AF.Square, accum_out=ssq[:])
                nm = sm.tile([U, 1], f32, tag=tag + "_nm")
                nc.vector.tensor_scalar_mul(nm[:], s[:], -1.0 / D)
                m2 = sm.tile([U, 1], f32, tag=tag + "_m2")
                nc.vector.tensor_mul(m2[:], nm[:], nm[:])
                vs2 = sm.tile([U, 1], f32, tag=tag + "_vs2")
                nc.vector.tensor_scalar(
                    vs2[:], ssq[:], 1.0 / D, EPS, op0=ALU.mult, op1=ALU.add
                )
                var = sm.tile([U, 1], f32, tag=tag + "_var")
                nc.vector.tensor_sub(var[:], vs2[:], m2[:])
                lv = sm.tile([U, 1], f32, tag=tag + "_lv")
                nc.scalar.activation(lv[:], var[:], AF.Ln)
                rstd = sm.tile([U, 1], f32, tag=tag + "_rs")
                nc.scalar.activation(rstd[:], lv[:], AF.Exp, scale=-0.5)
                nmr = sm.tile([U, 1], f32, tag=tag + "_nmr")
                nc.vector.tensor_mul(nmr[:], nm[:], rstd[:])
                xn = ap_.tile([U, D], out_dt, tag=tag)
                nc.vector.tensor_scalar(
                    xn[:], xin, rstd[:], nmr[:], op0=ALU.mult, op1=ALU.add
                )
                return xn

            def transpose_512(xn, ident, out_dt, psum_pool, tag):
                """(U, 512) token-major -> (128, KD, U) feature-major."""
                xT = ap_.tile([128, KD, U], out_dt, tag=tag)
                for c in range(KD):
                    tr = psum_pool.tile([128, U], xn.dtype, tag="ptr", bufs=2)
                    nc.tensor.transpose(
                        tr[:], xn[:, c * 128 : (c + 1) * 128], ident[0:U, 0:U]
                    )
                    nc.scalar.copy(xT[:, c, :], tr[:])
                return xT

            def proj_featmajor(psum_pool, w_sb, rhsT, bias_col, out_sb, oc, n):
                """out_sb[:, oc, :] (128, n) = (W.T @ rhsT-chunks) + bias.

                Feature-major output puts the bias on the partition dim, so
                it fuses into the PSUM-evacuation DVE op as a per-partition
                scalar add (no K=1 bias matmul, no LDWEIGHTS cost).
                """
                ps = psum_pool.tile([128, n], f32, tag="psS", bufs=4)
                for c in range(KD):
                    nc.tensor.matmul(
                        ps[:],
                        w_sb[:, c, oc * 128 : (oc + 1) * 128],
                        rhsT[:, c, :],
                        start=(c == 0),
                        stop=(c == KD - 1),
                    )
                nc.vector.tensor_scalar(
                    out_sb[:, oc, :], ps[:], bias_col[:, oc : oc + 1], None,
                    op0=ALU.add,
                )

            # ---- decoder stack ----
            x_cur = x0_sb
            with (
                tc.tile_pool(name="wts", bufs=2) as wp,
                tc.tile_pool(name="psd", bufs=1, space="PSUM") as psd,
            ):
                wout_sb = []
                for n in range(NOUT):
                    wt = jw.tile([128, KJ, 512], f32r, tag=f"wout{n}")
                    nc.gpsimd.dma_start(
                        wt[:],
                        wout_d.ap()[:, n * 512 : (n + 1) * 512].rearrange(
                            "(c p) n -> p c n", p=128
                        ),
                    )
                    wout_sb.append(wt)
                for l in range(L):
                    wq_sb = wp.tile([128, KD, D], bf16, tag="wq")
                    nc.sync.dma_start(
                        wq_sb[:], wq_d.ap()[l].rearrange("(c p) n -> p c n", p=128)
                    )
                    wk_sb = wp.tile([128, KD, D], bf16, tag="wk")
                    nc.sync.dma_start(
                        wk_sb[:], wk_d.ap()[l].rearrange("(c p) n -> p c n", p=128)
                    )
                    wv_sb = wp.tile([128, KD, D], bf16, tag="wv")
                    nc.sync.dma_start(
                        wv_sb[:], wv_d.ap()[l].rearrange("(c p) n -> p c n", p=128)
                    )
                    wo_sb = wp.tile([128, KD, D], bf16, tag="wo")
                    nc.sync.dma_start(
                        wo_sb[:], wo_d.ap()[l].rearrange("(c p) n -> p c n", p=128)
                    )
                    w1_sb = wp.tile([128, KD, DFF], bf16, tag="w1", bufs=1)
                    nc.sync.dma_start(
                        w1_sb[:], w1_d.ap()[l].rearrange("(c p) n -> p c n", p=128)
                    )
                    w2_sb = wp.tile([128, MD, D], bf16, tag="w2", bufs=1)
                    nc.sync.dma_start(
                        w2_sb[:], w2_d.ap()[l].rearrange("(c p) n -> p c n", p=128)
                    )
                    bq_sb = wp.tile([128, H], f32, tag="bq_l")
                    nc.gpsimd.dma_start(bq_sb[:], bq_d.ap()[l])
                    bk_sb = wp.tile([128, H], f32, tag="bk_l")
                    nc.gpsimd.dma_start(bk_sb[:], bk_d.ap()[l])
                    bv_sb = wp.tile([1, D], bf16, tag="bv_l")
                    nc.gpsimd.dma_start(bv_sb[:], bv_d.ap()[l : l + 1, :])
                    bo_sb = wp.tile([1, D], bf16, tag="bo_l")
                    nc.gpsimd.dma_start(bo_sb[:], bo_d.ap()[l : l + 1, :])
                    b1_sb = wp.tile([128, MD], f32, tag="b1_l")
                    nc.gpsimd.dma_start(b1_sb[:], b1_d.ap()[l])
                    b2_sb = wp.tile([1, D], bf16, tag="b2_l")
                    nc.gpsimd.dma_start(b2_sb[:], b2_d.ap()[l : l + 1, :])

                    # pre-norm attention
                    xn = layernorm(x_cur[:], bf16, "ln1")
                    hT = transpose_512(xn[:], identb, bf16, psd, "hT")
                    qT = ap_.tile([128, H, U], bf16, tag="qT")
                    kT = ap_.tile([128, H, U], bf16, tag="kT")
                    for h in range(H):
                        proj_featmajor(psd, wq_sb, hT, bq_sb, qT, h, U)
                        proj_featmajor(psd, wk_sb, hT, bk_sb, kT, h, U)
                    # v token-major
                    psv = psd.tile([U, D], f32, tag="psB", bufs=2)
                    for c in range(KD):
                        nc.tensor.matmul(
                            psv[:],
                            hT[:, c, :],
                            wv_sb[:, c, :],
                            start=(c == 0),
                            stop=False,
                        )
                    nc.tensor.matmul(
                        psv[:], onesb[0:1, 0:U], bv_sb[0:1, :], start=False, stop=True
                    )
                    v_sb = ap_.tile([U, D], bf16, tag="v")
                    nc.vector.tensor_copy(v_sb[:], psv[:])

                    # scores (q-major), mask, softmax
                    ps_sc = psd.tile([U, H * U], f32, tag="psB", bufs=2)
                    for h in range(H):
                        nc.tensor.matmul(
                            ps_sc[:, h * U : (h + 1) * U],
                            qT[:, h, :],
                            kT[:, h, :],
                            start=True,
                            stop=True,
                        )
                    esc = ap_.tile([U, H * U], f32, tag="esc")
                    nc.vector.tensor_add(esc[:], ps_sc[:], mask_sb[:])
                    escx = ap_.tile([U, H, U], f32, tag="escx")
                    ssum = sm.tile([U, H], f32, tag="ssum")
                    for h in range(H):
                        nc.scalar.activation(
                            escx[:, h, :],
                            esc[:, h * U : (h + 1) * U],
                            AF.Exp,
                            accum_out=ssum[:, h : h + 1],
                        )
                    rec = sm.tile([U, H], f32, tag="rec")
                    nc.vector.reciprocal(rec[:], ssum[:])
                    attnT = ap_.tile([U, H, U], bf16, tag="attnT")
                    for h in range(H):
                        an = ap_.tile([U, U], bf16, tag="an")
                        nc.vector.tensor_scalar_mul(
                            an[:], escx[:, h, :], rec[:, h : h + 1]
                        )
                        pt = psd.tile([U, U], bf16, tag="psS", bufs=4)
                        nc.tensor.transpose(pt[:], an[:], identb[0:U, 0:U])
                        nc.scalar.copy(attnT[:, h, :], pt[:])
                    # oT feature-major: oT[h] = v[h].T @ attnT[h]
                    oT = ap_.tile([128, H, U], bf16, tag="oT")
                    for h in range(H):
                        po = psd.tile([128, U], f32, tag="psS", bufs=4)
                        nc.tensor.matmul(
                            po[:],
                            v_sb[:, h * 128 : (h + 1) * 128],
                            attnT[:, h, :],
                            start=True,
                            stop=True,
                        )
                        nc.scalar.copy(oT[:, h, :], po[:])
                    # x += o @ Wo + bo
                    px = psd.tile([U, D], f32, tag="psB", bufs=2)
                    for h in range(H):
                        nc.tensor.matmul(
                            px[:], oT[:, h, :], wo_sb[:, h, :], start=(h == 0),
                            stop=False,
                        )
                    nc.tensor.matmul(
                        px[:], onesb[0:1, 0:U], bo_sb[0:1, :], start=False, stop=True
                    )
                    x_att = ap_.tile([U, D], f32, tag="x")
                    s_att = sm.tile([U, 1], f32, tag="s_att")
                    nc.vector.scalar_tensor_tensor(
                        x_att[:], px[:], 1.0, x_cur[:],
                        op0=ALU.mult, op1=ALU.add, accum_out=s_att[:],
                    )

                    # FFN
                    x2n = layernorm(x_att[:], bf16, "ln2", s_pre=s_att)
                    h2T = transpose_512(x2n[:], identb, bf16, psd, "h2T")
                    zT = ap_.tile([128, MD, U], bf16, tag="zT")
                    for m in range(MD):
                        pz = psd.tile([128, U], f32, tag="psS", bufs=4)
                        for c in range(KD):
                            nc.tensor.matmul(
                                pz[:],
                                w1_sb[:, c, m * 128 : (m + 1) * 128],
                                h2T[:, c, :],
                                start=(c == 0),
                                stop=(c == KD - 1),
                            )
                        nc.scalar.activation(
                            zT[:, m, :], pz[:], AF.Relu, bias=b1_sb[:, m : m + 1]
                        )
                    px2 = psd.tile([U, D], f32, tag="psB", bufs=2)
                    for m in range(MD):
                        nc.tensor.matmul(
                            px2[:],
                            zT[:, m, :],
                            w2_sb[:, m, :],
                            start=(m == 0),
                            stop=False,
                        )
                    nc.tensor.matmul(
                        px2[:], onesb[0:1, 0:U], b2_sb[0:1, :], start=False, stop=True
                    )
                    x_new = ap_.tile([U, D], f32, tag="x")
                    s_new = sm.tile([U, 1], f32, tag="s_new")
                    nc.vector.scalar_tensor_tensor(
                        x_new[:], px2[:], 1.0, x_att[:],
                        op0=ALU.mult, op1=ALU.add, accum_out=s_new[:],
                    )
                    x_cur = x_new
                    s_cur = s_new

                # joint encoder-side projection (feature-major)
                nc.sync.dma_start(
                    memT_sb[:], memT_d.ap().rearrange("(c p) t -> p c t", p=128)
                )
                nc.sync.dma_start(identr[:], identr_d.ap())
                nc.sync.dma_start(
                    wenc_sb[:], wenc_d.ap().rearrange("(c p) n -> p c n", p=128)
                )
                nc.sync.dma_start(
                    wdec_sb[:], wdec_d.ap().rearrange("(c p) n -> p c n", p=128)
                )
                for jc in range(KJ):
                    proj_featmajor(
                        psd, wenc_sb, memT_sb, benc_sb, zencT, jc, TS
                    )
                # final layernorm (gains folded into wdec) + z_decT
                xf = layernorm(x_cur[:], f32r, "lnf", s_pre=s_cur)
                xfT = transpose_512(xf[:], identr, f32r, psd, "xfT")
                for jc in range(KJ):
                    proj_featmajor(
                        psd, wdec_sb, xfT, bdec_sb, zdecT, jc, TS
                    )

            # ---- joint network ----
            with (
                tc.tile_pool(name="jact", bufs=2) as ja,
                tc.tile_pool(name="psj", bufs=6, space="PSUM") as psj,
            ):
                boutr_sb = ja.tile([128, ODIM], f32, tag="boutr", bufs=1)
                nc.sync.dma_start(boutr_sb[:], boutr_d.ap())
                for i in range(TS // 2):
                    zt = ja.tile([128, KJ, 2 * U], f32r, tag="zt")
                    for jc in range(KJ):
                        for tt in range(2):
                            nc.scalar.activation(
                                zt[:, jc, tt * U : (tt + 1) * U],
                                zdecT[:, jc, :],
                                AF.Tanh,
                                bias=zencT[:, jc, 2 * i + tt : 2 * i + tt + 1],
                            )
                    ob = ja.tile([128, ODIM], f32, tag="ob")
                    for n in range(NOUT):
                        pj = psj.tile([128, 512], f32, tag="pj")
                        for jc in range(KJ):
                            nc.tensor.matmul(
                                pj[:],
                                zt[:, jc, :],
                                wout_sb[n][:, jc, :],
                                start=(jc == 0),
                                stop=(jc == KJ - 1),
                            )
                        nc.vector.tensor_add(
                            ob[:, n * 512 : (n + 1) * 512],
                            pj[:],
                            boutr_sb[:, n * 512 : (n + 1) * 512],
                        )
                    nc.sync.dma_start(out_d.ap()[i * 128 : (i + 1) * 128, :], ob[:])

    nc.compile()
    return nc


def _get_nc():
    if "nc" not in _CACHE:
        _CACHE["nc"] = _build()
    return _CACHE["nc"]


def _pos_enc():
    pos = np.arange(U, dtype=np.float32)[:, None]
    div = np.exp(
        np.arange(0, D, 2, dtype=np.float32) * (-math.log(10000.0) / D)
    ).astype(np.float32)
    pe = np.zeros((U, D), dtype=np.float32)
    pe[:, 0::2] = np.sin(pos * div)
    pe[:, 1::2] = np.cos(pos * div)
    return pe


def _prep_maps(inputs):
    import ml_dtypes

    f = np.float32
    bf = ml_dtypes.bfloat16
    tgt = np.asarray(inputs["tgt"])
    tgt_mask = np.asarray(inputs["tgt_mask"])
    memory = np.asarray(inputs["memory"], dtype=f)
    embed = np.asarray(inputs["embed"], dtype=f)
    g1 = np.asarray(inputs["ln1_g"], np.float64)
    c1 = np.asarray(inputs["ln1_b"], np.float64)
    g2 = np.asarray(inputs["ln2_g"], np.float64)
    c2 = np.asarray(inputs["ln2_b"], np.float64)
    gf = np.asarray(inputs["lnf_g"], np.float64)
    cf = np.asarray(inputs["lnf_b"], np.float64)
    Wq = np.asarray(inputs["Wq"], np.float64)
    Wk = np.asarray(inputs["Wk"], np.float64)
    Wv = np.asarray(inputs["Wv"], np.float64)
    Wo = np.asarray(inputs["Wo"], f)
    bq = np.asarray(inputs["bq"], np.float64)
    bk = np.asarray(inputs["bk"], np.float64)
    bv = np.asarray(inputs["bv"], np.float64)
    bo = np.asarray(inputs["bo"], f)
    W1 = np.asarray(inputs["W1"], np.float64)
    b1 = np.asarray(inputs["b1"], np.float64)
    W2 = np.asarray(inputs["W2"], f)
    b2 = np.asarray(inputs["b2"], f)
    W_enc = np.asarray(inputs["W_enc"], f)
    b_enc = np.asarray(inputs["b_enc"], f)
    W_dec = np.asarray(inputs["W_dec"], np.float64)
    W_out = np.asarray(inputs["W_out"], f)
    b_out = np.asarray(inputs["b_out"], f)

    s = 1.0 / math.sqrt(DH)
    wq_f = np.empty((L, D, D), bf)
    wk_f = np.empty((L, D, D), bf)
    wv_f = np.empty((L, D, D), bf)
    w1_f = np.empty((L, D, DFF), bf)
    bq_f = np.empty((L, 128, H), f)
    bk_f = np.empty((L, 128, H), f)
    bv_f = np.empty((L, D), bf)
    b1_f = np.empty((L, 128, MD), f)
    for l in range(L):
        wq_f[l] = (g1[l][:, None] * Wq[l] * s).astype(bf)
        bq_f[l] = ((bq[l] + c1[l] @ Wq[l]) * s).astype(f).reshape(H, 128).T
        wk_f[l] = (g1[l][:, None] * Wk[l]).astype(bf)
        bk_f[l] = (bk[l] + c1[l] @ Wk[l]).astype(f).reshape(H, 128).T
        wv_f[l] = (g1[l][:, None] * Wv[l]).astype(bf)
        bv_f[l] = (bv[l] + c1[l] @ Wv[l]).astype(bf)
        w1_f[l] = (g2[l][:, None] * W1[l]).astype(bf)
        b1_f[l] = (b1[l] + c2[l] @ W1[l]).astype(f).reshape(MD, 128).T
    wdec_f = (gf[:, None] * W_dec).astype(f)
    bdec_f = np.ascontiguousarray((cf @ W_dec).astype(f).reshape(KJ, 128).T)

    x0_all = (embed[tgt] * np.float32(math.sqrt(D)) + _pos_enc()[None]).astype(f)
    boutr = np.broadcast_to(b_out, (128, ODIM)).copy()

    common = dict(
        identb=np.eye(128, dtype=bf), identr=np.eye(128, dtype=f),
        onesb=np.ones((1, 128), dtype=bf), onesr=np.ones((1, 128), dtype=f),
        wq=wq_f, wk=wk_f, wv=wv_f, wo=Wo.astype(bf), w1=w1_f, w2=W2.astype(bf),
        bq=bq_f, bk=bk_f, bv=bv_f, bo=bo.astype(bf), b1=b1_f, b2=b2.astype(bf),
        wenc=W_enc, benc=b_enc.astype(f).reshape(1, J),
        wdec=wdec_f, bdec=bdec_f,
        wout=W_out, boutr=boutr,
    )
    in_maps = []
    for c in range(NCORES):
        b, th = divmod(c, 2)
        mk = np.where(tgt_mask[b], f(0.0), f(-1e9)).astype(f)  # (U, U)
        m = dict(common)
        m["x0"] = x0_all[b]
        m["memT"] = np.ascontiguousarray(memory[b, th * TS : (th + 1) * TS, :].T)
        m["maskb"] = np.ascontiguousarray(np.tile(mk, (1, H)))
        in_maps.append(m)
    return in_maps, tgt_mask


def _gather(results):
    z = np.empty((B, T, U, ODIM), np.float32)
    for c in range(NCORES):
        b, th = divmod(c, 2)
        z[b, th * TS : (th + 1) * TS] = results[c]["out"].reshape(TS, U, ODIM)
    return z


def kernel(**inputs):
    from concourse.bass_utils import run_bass_kernel_spmd

    nc = _get_nc()
    in_maps, tgt_mask = _prep_maps(inputs)
    res = run_bass_kernel_spmd(nc, in_maps, list(range(NCORES)))
    return _gather(res.results), tgt_mask


def run_traced(inputs):
    """For test.py: returns (z, tgt_mask, BassKernelResults with timing)."""
    from concourse.bass_utils import run_bass_kernel_spmd

    nc = _get_nc()
    in_maps, tgt_mask = _prep_maps(inputs)
    res = run_bass_kernel_spmd(nc, in_maps, list(range(NCORES)), trace=True)
    return _gather(res.results), tgt_mask, res


# revision 16
# speedup vs baseline: 1.1251x; 1.1251x over previous
"""Trainium2 Bass kernel for nn_Decoder_33088428049154.

Sharding: 8 cores; core c handles batch b=c//2 and T-half th=c%2.
Each core runs the full 6-layer decoder for its batch element (replicated
across the pair) and the joint network for its (64 t) x (64 u) x 4096 slice.

Host-side prep (inside kernel()): embedding gather + positional encoding,
causal-mask -> additive bias, LN gain/bias folded into the following weight
matrices, 1/sqrt(dh) folded into Wq, memory slice transposes, weight casts.

Device: decoder matmuls in bf16 (FWL weight loads, half DMA) with the
residual stream / layernorm stats / softmax kept in fp32; joint network in
float32r (full PE rate at N=512, ~1e-4 relative error). Joint z built
feature-major so tanh output directly feeds lhsT of the
(4096 rows x 512 K x 4096 N) output matmul; its bias is fused into the
PSUM-evacuation vector add via a host-replicated b_out.
"""

import math

import numpy as np

B, T, U = 4, 128, 64
D, H, DFF, J, ODIM, L = 512, 4, 2048, 512, 4096, 6
DH = D // H  # 128
KD = D // 128  # 4
KJ = J // 128  # 4
MD = DFF // 128  # 16
NOUT = ODIM // 512  # 8
TS = T // 2  # 64 timesteps per core
NCORES = 8
EPS = 1e-12

_CACHE = {}


def _build():
    import concourse.mybir as mybir
    from concourse import bacc, tile

    f32 = mybir.dt.float32
    f32r = mybir.dt.float32r
    bf16 = mybir.dt.bfloat16
    AF = mybir.ActivationFunctionType
    ALU = mybir.AluOpType
    AX = mybir.AxisListType

    nc = bacc.Bacc("TRN2", target_bir_lowering=False, debug=False, num_devices=8)

    # ---- DRAM I/O ----
    x0_d = nc.dram_tensor("x0", [U, D], f32, kind="ExternalInput")
    memT_d = nc.dram_tensor("memT", [D, TS], f32r, kind="ExternalInput")
    mask_d = nc.dram_tensor("maskb", [U, H * U], f32, kind="ExternalInput")
    identb_d = nc.dram_tensor("identb", [128, 128], bf16, kind="ExternalInput")
    identr_d = nc.dram_tensor("identr", [128, 128], f32r, kind="ExternalInput")
    onesb_d = nc.dram_tensor("onesb", [1, 128], bf16, kind="ExternalInput")
    onesr_d = nc.dram_tensor("onesr", [1, 128], f32r, kind="ExternalInput")
    wq_d = nc.dram_tensor("wq", [L, D, D], bf16, kind="ExternalInput")
    wk_d = nc.dram_tensor("wk", [L, D, D], bf16, kind="ExternalInput")
    wv_d = nc.dram_tensor("wv", [L, D, D], bf16, kind="ExternalInput")
    wo_d = nc.dram_tensor("wo", [L, D, D], bf16, kind="ExternalInput")
    w1_d = nc.dram_tensor("w1", [L, D, DFF], bf16, kind="ExternalInput")
    w2_d = nc.dram_tensor("w2", [L, DFF, D], bf16, kind="ExternalInput")
    bq_d = nc.dram_tensor("bq", [L, 128, H], f32, kind="ExternalInput")
    bk_d = nc.dram_tensor("bk", [L, 128, H], f32, kind="ExternalInput")
    bv_d = nc.dram_tensor("bv", [L, D], bf16, kind="ExternalInput")
    bo_d = nc.dram_tensor("bo", [L, D], bf16, kind="ExternalInput")
    b1_d = nc.dram_tensor("b1", [L, 128, MD], f32, kind="ExternalInput")
    b2_d = nc.dram_tensor("b2", [L, D], bf16, kind="ExternalInput")
    wenc_d = nc.dram_tensor("wenc", [D, J], f32r, kind="ExternalInput")
    benc_d = nc.dram_tensor("benc", [128, KJ], f32, kind="ExternalInput")
    wdec_d = nc.dram_tensor("wdec", [D, J], f32r, kind="ExternalInput")
    bdec_d = nc.dram_tensor("bdec", [128, KJ], f32, kind="ExternalInput")
    wout_d = nc.dram_tensor("wout", [J, ODIM], f32r, kind="ExternalInput")
    boutr_d = nc.dram_tensor("boutr", [128, ODIM], f32, kind="ExternalInput")
    out_d = nc.dram_tensor("out", [TS * U, ODIM], f32, kind="ExternalOutput")

    with tile.TileContext(nc) as tc:
        with (
            tc.tile_pool(name="const", bufs=1) as cpool,
            tc.tile_pool(name="bias", bufs=1) as bpool,
            tc.tile_pool(name="persist", bufs=1) as ppool,
            tc.tile_pool(name="sm", bufs=3) as sm,
            tc.tile_pool(name="act", bufs=2) as ap_,
            tc.tile_pool(name="jw", bufs=1) as jw,
        ):
            identb = cpool.tile([128, 128], bf16, tag="identb")
            identr = cpool.tile([128, 128], f32r, tag="identr")
            onesb = cpool.tile([1, 128], bf16, tag="onesb")
            onesr = cpool.tile([1, 128], f32r, tag="onesr")

            benc_sb = bpool.tile([128, KJ], f32, tag="benc")
            nc.gpsimd.dma_start(benc_sb[:], benc_d.ap())
            bdec_sb = bpool.tile([128, KJ], f32, tag="bdec")
            nc.gpsimd.dma_start(bdec_sb[:], bdec_d.ap())

            x0_sb = ppool.tile([U, D], f32, tag="x0")
            nc.gpsimd.dma_start(x0_sb[:], x0_d.ap())
            nc.gpsimd.dma_start(identb[:], identb_d.ap())
            nc.gpsimd.dma_start(onesb[:], onesb_d.ap())
            nc.gpsimd.dma_start(onesr[:], onesr_d.ap())
            mask_sb = ppool.tile([U, H * U], f32, tag="mask")
            nc.gpsimd.dma_start(mask_sb[:], mask_d.ap())
            memT_sb = ppool.tile([128, KD, TS], f32r, tag="memT")
            wenc_sb = ppool.tile([128, KD, J], f32r, tag="wenc")
            wdec_sb = ppool.tile([128, KD, J], f32r, tag="wdec")
            zencT = ppool.tile([128, KJ, TS], f32, tag="zencT")
            zdecT = ppool.tile([128, KJ, TS], f32, tag="zdecT")

            def layernorm(xin, out_dt, tag, s_pre=None):
                """Return xn = (xin - mean) / sqrt(var + eps), token-major.

                var = E[x^2] - mean^2 so the Square/accum (ACT) runs in
                parallel with the sum reduce (DVE); s_pre is a precomputed
                row-sum (from a fused residual-add accum_out) if available.
                """
                if s_pre is None:
                    s = sm.tile([U, 1], f32, tag=tag + "_s")
                    nc.vector.tensor_reduce(s[:], xin, axis=AX.X, op=ALU.add)
                else:
                    s = s_pre
                sq = ap_.tile([U, D], f32, tag="lnsq")
                ssq = sm.tile([U, 1], f32, tag=tag + "_ssq")
                nc.scalar.activation(sq[:], xin, AF.Square, accum_out=ssq[:])
                nm = sm.tile([U, 1], f32, tag=tag + "_nm")
                nc.vector.tensor_scalar_mul(nm[:], s[:], -1.0 / D)
                m2 = sm.tile([U, 1], f32, tag=tag + "_m2")
                nc.vector.tensor_mul(m2[:], nm[:], nm[:])
                vs2 = sm.tile([U, 1], f32, tag=tag + "_vs2")
                nc.vector.tensor_scalar(
                    vs2[:], ssq[:], 1.0 / D, EPS, op0=ALU.mult, op1=ALU.add
                )
                var = sm.tile([U, 1], f32, tag=tag + "_var")
                nc.vector.tensor_sub(var[:], vs2[:], m2[:])
                sd = sm.tile([U, 1], f32, tag=tag + "_sd")
                nc.scalar.activation(sd[:], var[:], AF.Sqrt)
                rstd = sm.tile([U, 1], f32, tag=tag + "_rs")
                nc.vector.reciprocal(rstd[:], sd[:])
                nmr = sm.tile([U, 1], f32, tag=tag + "_nmr")
                nc.vector.tensor_mul(nmr[:], nm[:], rstd[:])
                xn = ap_.tile([U, D], out_dt, tag=tag)
                nc.vector.tensor_scalar(
                    xn[:], xin, rstd[:], nmr[:], op0=ALU.mult, op1=ALU.add
                )
                return xn

            def transpose_512(xn, ident, out_dt, psum_pool, tag):
                """(U, 512) token-major -> (128, KD, U) feature-major."""
                xT = ap_.tile([128, KD, U], out_dt, tag=tag)
                for c in range(KD):
                    tr = psum_pool.tile([128, U], xn.dtype, tag="ptr", bufs=2)
                    nc.tensor.transpose(
                        tr[:], xn[:, c * 128 : (c + 1) * 128], ident[0:U, 0:U]
                    )
                    nc.scalar.copy(xT[:, c, :], tr[:])
                return xT

            def proj_featmajor(psum_pool, w_sb, rhsT, bias_col, out_sb, oc, n):
                """out_sb[:, oc, :] (128, n) = (W.T @ rhsT-chunks) + bias.

                Feature-major output puts the bias on the partition dim, so
                it fuses into the PSUM-evacuation DVE op as a per-partition
                scalar add (no K=1 bias matmul, no LDWEIGHTS cost).
                """
                ps = psum_pool.tile([128, n], f32, tag="psS", bufs=4)
                for c in range(KD):
                    nc.tensor.matmul(
                        ps[:],
                        w_sb[:, c, oc * 128 : (oc + 1) * 128],
                        rhsT[:, c, :],
                        start=(c == 0),
                        stop=(c == KD - 1),
                    )
                nc.vector.tensor_scalar(
                    out_sb[:, oc, :], ps[:], bias_col[:, oc : oc + 1], None,
                    op0=ALU.add,
                )

            # ---- decoder stack ----
            x_cur = x0_sb
            with (
                tc.tile_pool(name="wts", bufs=2) as wp,
                tc.tile_pool(name="psd", bufs=1, space="PSUM") as psd,
            ):
                wout_sb = []
                for l in range(L):
                    wq_sb = wp.tile([128, KD, D], bf16, tag="wq")
                    nc.sync.dma_start(
                        wq_sb[:], wq_d.ap()[l].rearrange("(c p) n -> p c n", p=128)
                    )
                    wk_sb = wp.tile([128, KD, D], bf16, tag="wk")
                    nc.sync.dma_start(
                        wk_sb[:], wk_d.ap()[l].rearrange("(c p) n -> p c n", p=128)
                    )
                    wv_sb = wp.tile([128, KD, D], bf16, tag="wv")
                    nc.sync.dma_start(
                        wv_sb[:], wv_d.ap()[l].rearrange("(c p) n -> p c n", p=128)
                    )
                    wo_sb = wp.tile([128, KD, D], bf16, tag="wo")
                    nc.sync.dma_start(
                        wo_sb[:], wo_d.ap()[l].rearrange("(c p) n -> p c n", p=128)
                    )
                    w1_sb = wp.tile([128, KD, DFF], bf16, tag="w1", bufs=1)
                    nc.sync.dma_start(
                        w1_sb[:], w1_d.ap()[l].rearrange("(c p) n -> p c n", p=128)
                    )
                    w2_sb = wp.tile([128, MD, D], bf16, tag="w2", bufs=1)
                    nc.sync.dma_start(
                        w2_sb[:], w2_d.ap()[l].rearrange("(c p) n -> p c n", p=128)
                    )
                    bq_sb = wp.tile([128, H], f32, tag="bq_l")
                    nc.gpsimd.dma_start(bq_sb[:], bq_d.ap()[l])
                    bk_sb = wp.tile([128, H], f32, tag="bk_l")
                    nc.gpsimd.dma_start(bk_sb[:], bk_d.ap()[l])
                    bv_sb = wp.tile([1, D], bf16, tag="bv_l")
                    nc.gpsimd.dma_start(bv_sb[:], bv_d.ap()[l : l + 1, :])
                    bo_sb = wp.tile([1, D], bf16, tag="bo_l")
                    nc.gpsimd.dma_start(bo_sb[:], bo_d.ap()[l : l + 1, :])
                    b1_sb = wp.tile([128, MD], f32, tag="b1_l")
                    nc.gpsimd.dma_start(b1_sb[:], b1_d.ap()[l])
                    b2_sb = wp.tile([1, D], bf16, tag="b2_l")
                    nc.gpsimd.dma_start(b2_sb[:], b2_d.ap()[l : l + 1, :])
                    if l == 2:
                        for n in range(NOUT):
                            wt = jw.tile([128, KJ, 512], f32r, tag=f"wout{n}")
                            nc.sync.dma_start(
                                wt[:],
                                wout_d.ap()[:, n * 512 : (n + 1) * 512].rearrange(
                                    "(c p) n -> p c n", p=128
                                ),
                            )
                            wout_sb.append(wt)

                    # pre-norm attention
                    xn = layernorm(x_cur[:], bf16, "ln1")
                    hT = transpose_512(xn[:], identb, bf16, psd, "hT")
                    qT = ap_.tile([128, H, U], bf16, tag="qT")
                    kT = ap_.tile([128, H, U], bf16, tag="kT")
                    for h in range(H):
                        proj_featmajor(psd, wq_sb, hT, bq_sb, qT, h, U)
                        proj_featmajor(psd, wk_sb, hT, bk_sb, kT, h, U)
                    # v token-major
                    psv = psd.tile([U, D], f32, tag="psB", bufs=2)
                    for c in range(KD):
                        nc.tensor.matmul(
                            psv[:],
                            hT[:, c, :],
                            wv_sb[:, c, :],
                            start=(c == 0),
                            stop=False,
                        )
                    nc.tensor.matmul(
                        psv[:], onesb[0:1, 0:U], bv_sb[0:1, :], start=False, stop=True
                    )
                    v_sb = ap_.tile([U, D], bf16, tag="v")
                    nc.vector.tensor_copy(v_sb[:], psv[:])

                    # scores (q-major), mask, softmax
                    ps_sc = psd.tile([U, H * U], f32, tag="psB", bufs=2)
                    for h in range(H):
                        nc.tensor.matmul(
                            ps_sc[:, h * U : (h + 1) * U],
                            qT[:, h, :],
                            kT[:, h, :],
                            start=True,
                            stop=True,
                        )
                    esc = ap_.tile([U, H * U], f32, tag="esc")
                    nc.vector.tensor_add(esc[:], ps_sc[:], mask_sb[:])
                    escx = ap_.tile([U, H, U], f32, tag="escx")
                    ssum = sm.tile([U, H], f32, tag="ssum")
                    for h in range(H):
                        nc.scalar.activation(
                            escx[:, h, :],
                            esc[:, h * U : (h + 1) * U],
                            AF.Exp,
                            accum_out=ssum[:, h : h + 1],
                        )
                    rec = sm.tile([U, H], f32, tag="rec")
                    nc.vector.reciprocal(rec[:], ssum[:])
                    attnT = ap_.tile([U, H, U], bf16, tag="attnT")
                    for h in range(H):
                        an = ap_.tile([U, U], bf16, tag="an")
                        nc.vector.tensor_scalar_mul(
                            an[:], escx[:, h, :], rec[:, h : h + 1]
                        )
                        pt = psd.tile([U, U], bf16, tag="psS", bufs=4)
                        nc.tensor.transpose(pt[:], an[:], identb[0:U, 0:U])
                        nc.scalar.copy(attnT[:, h, :], pt[:])
                    # oT feature-major: oT[h] = v[h].T @ attnT[h]
                    oT = ap_.tile([128, H, U], bf16, tag="oT")
                    for h in range(H):
                        po = psd.tile([128, U], f32, tag="psS", bufs=4)
                        nc.tensor.matmul(
                            po[:],
                            v_sb[:, h * 128 : (h + 1) * 128],
                            attnT[:, h, :],
                            start=True,
                            stop=True,
                        )
                        nc.scalar.copy(oT[:, h, :], po[:])
                    # x += o @ Wo + bo
                    px = psd.tile([U, D], f32, tag="psB", bufs=2)
                    for h in range(H):
                        nc.tensor.matmul(
                            px[:], oT[:, h, :], wo_sb[:, h, :], start=(h == 0),
                            stop=False,
                        )
                    nc.tensor.matmul(
                        px[:], onesb[0:1, 0:U], bo_sb[0:1, :], start=False, stop=True
                    )
                    x_att = ap_.tile([U, D], f32, tag="x")
                    s_att = sm.tile([U, 1], f32, tag="s_att")
                    nc.vector.scalar_tensor_tensor(
                        x_att[:], px[:], 1.0, x_cur[:],
                        op0=ALU.mult, op1=ALU.add, accum_out=s_att[:],
                    )

                    # FFN
                    x2n = layernorm(x_att[:], bf16, "ln2", s_pre=s_att)
                    h2T = transpose_512(x2n[:], identb, bf16, psd, "h2T")
                    zT = ap_.tile([128, MD, U], bf16, tag="zT")
                    for m in range(MD):
                        pz = psd.tile([128, U], f32, tag="psS", bufs=4)
                        for c in range(KD):
                            nc.tensor.matmul(
                                pz[:],
                                w1_sb[:, c, m * 128 : (m + 1) * 128],
                                h2T[:, c, :],
                                start=(c == 0),
                                stop=(c == KD - 1),
                            )
                        nc.scalar.activation(
                            zT[:, m, :], pz[:], AF.Relu, bias=b1_sb[:, m : m + 1]
                        )
                    px2 = psd.tile([U, D], f32, tag="psB", bufs=2)
                    for m in range(MD):
                        nc.tensor.matmul(
                            px2[:],
                            zT[:, m, :],
                            w2_sb[:, m, :],
                            start=(m == 0),
                            stop=False,
                        )
                    nc.tensor.matmul(
                        px2[:], onesb[0:1, 0:U], b2_sb[0:1, :], start=False, stop=True
                    )
                    x_new = ap_.tile([U, D], f32, tag="x")
                    s_new = sm.tile([U, 1], f32, tag="s_new")
                    nc.vector.scalar_tensor_tensor(
                        x_new[:], px2[:], 1.0, x_att[:],
                        op0=ALU.mult, op1=ALU.add, accum_out=s_new[:],
                    )
                    x_cur = x_new
                    s_cur = s_new

                # joint encoder-side projection (feature-major)
                nc.sync.dma_start(
                    memT_sb[:], memT_d.ap().rearrange("(c p) t -> p c t", p=128)
                )
                nc.sync.dma_start(identr[:], identr_d.ap())
                nc.sync.dma_start(
                    wenc_sb[:], wenc_d.ap().rearrange("(c p) n -> p c n", p=128)
                )
                nc.sync.dma_start(
                    wdec_sb[:], wdec_d.ap().rearrange("(c p) n -> p c n", p=128)
                )
                for jc in range(KJ):
                    proj_featmajor(
                        psd, wenc_sb, memT_sb, benc_sb, zencT, jc, TS
                    )
                # final layernorm (gains folded into wdec) + z_decT
                xf = layernorm(x_cur[:], f32r, "lnf", s_pre=s_cur)
                xfT = transpose_512(xf[:], identr, f32r, psd, "xfT")
                for jc in range(KJ):
                    proj_featmajor(
                        psd, wdec_sb, xfT, bdec_sb, zdecT, jc, TS
                    )

            # ---- joint network ----
            with (
                tc.tile_pool(name="jact", bufs=2) as ja,
                tc.tile_pool(name="psj", bufs=6, space="PSUM") as psj,
            ):
                boutr_sb = ja.tile([128, ODIM], f32, tag="boutr", bufs=1)
                nc.sync.dma_start(boutr_sb[:], boutr_d.ap())
                for i in range(TS // 2):
                    zt = ja.tile([128, KJ, 2 * U], f32r, tag="zt")
                    for jc in range(KJ):
                        for tt in range(2):
                            nc.scalar.activation(
                                zt[:, jc, tt * U : (tt + 1) * U],
                                zdecT[:, jc, :],
                                AF.Tanh,
                                bias=zencT[:, jc, 2 * i + tt : 2 * i + tt + 1],
                            )
                    ob = ja.tile([128, ODIM], f32, tag="ob")
                    for n in range(NOUT):
                        pj = psj.tile([128, 512], f32, tag="pj")
                        for jc in range(KJ):
                            nc.tensor.matmul(
                                pj[:],
                                zt[:, jc, :],
                                wout_sb[n][:, jc, :],
                                start=(jc == 0),
                                stop=(jc == KJ - 1),
                            )
                        nc.vector.tensor_add(
                            ob[:, n * 512 : (n + 1) * 512],
                            pj[:],
                            boutr_sb[:, n * 512 : (n + 1) * 512],
                        )
                    nc.sync.dma_start(out_d.ap()[i * 128 : (i + 1) * 128, :], ob[:])

    nc.compile()
    return nc


def _get_nc():
    if "nc" not in _CACHE:
        _CACHE["nc"] = _build()
    return _CACHE["nc"]


def _pos_enc():
    pos = np.arange(U, dtype=np.float32)[:, None]
    div = np.exp(
        np.arange(0, D, 2, dtype=np.float32) * (-math.log(10000.0) / D)
    ).astype(np.float32)
    pe = np.zeros((U, D), dtype=np.float32)
    pe[:, 0::2] = np.sin(pos * div)
    pe[:, 1::2] = np.cos(pos * div)
    return pe


def _prep_maps(inputs):
    import ml_dtypes

    f = np.float32
    bf = ml_dtypes.bfloat16
    tgt = np.asarray(inputs["tgt"])
    tgt_mask = np.asarray(inputs["tgt_mask"])
    memory = np.asarray(inputs["memory"], dtype=f)
    embed = np.asarray(inputs["embed"], dtype=f)
    g1 = np.asarray(inputs["ln1_g"], np.float64)
    c1 = np.asarray(inputs["ln1_b"], np.float64)
    g2 = np.asarray(inputs["ln2_g"], np.float64)
    c2 = np.asarray(inputs["ln2_b"], np.float64)
    gf = np.asarray(inputs["lnf_g"], np.float64)
    cf = np.asarray(inputs["lnf_b"], np.float64)
    Wq = np.asarray(inputs["Wq"], np.float64)
    Wk = np.asarray(inputs["Wk"], np.float64)
    Wv = np.asarray(inputs["Wv"], np.float64)
    Wo = np.asarray(inputs["Wo"], f)
    bq = np.asarray(inputs["bq"], np.float64)
    bk = np.asarray(inputs["bk"], np.float64)
    bv = np.asarray(inputs["bv"], np.float64)
    bo = np.asarray(inputs["bo"], f)
    W1 = np.asarray(inputs["W1"], np.float64)
    b1 = np.asarray(inputs["b1"], np.float64)
    W2 = np.asarray(inputs["W2"], f)
    b2 = np.asarray(inputs["b2"], f)
    W_enc = np.asarray(inputs["W_enc"], f)
    b_enc = np.asarray(inputs["b_enc"], f)
    W_dec = np.asarray(inputs["W_dec"], np.float64)
    W_out = np.asarray(inputs["W_out"], f)
    b_out = np.asarray(inputs["b_out"], f)

    s = 1.0 / math.sqrt(DH)
    wq_f = np.empty((L, D, D), bf)
    wk_f = np.empty((L, D, D), bf)
    wv_f = np.empty((L, D, D), bf)
    w1_f = np.empty((L, D, DFF), bf)
    bq_f = np.empty((L, 128, H), f)
    bk_f = np.empty((L, 128, H), f)
    bv_f = np.empty((L, D), bf)
    b1_f = np.empty((L, 128, MD), f)
    for l in range(L):
        wq_f[l] = (g1[l][:, None] * Wq[l] * s).astype(bf)
        bq_f[l] = ((bq[l] + c1[l] @ Wq[l]) * s).astype(f).reshape(H, 128).T
        wk_f[l] = (g1[l][:, None] * Wk[l]).astype(bf)
        bk_f[l] = (bk[l] + c1[l] @ Wk[l]).astype(f).reshape(H, 128).T
        wv_f[l] = (g1[l][:, None] * Wv[l]).astype(bf)
        bv_f[l] = (bv[l] + c1[l] @ Wv[l]).astype(bf)
        w1_f[l] = (g2[l][:, None] * W1[l]).astype(bf)
        b1_f[l] = (b1[l] + c2[l] @ W1[l]).astype(f).reshape(MD, 128).T
    wdec_f = (gf[:, None] * W_dec).astype(f)
    bdec_f = np.ascontiguousarray((cf @ W_dec).astype(f).reshape(KJ, 128).T)

    x0_all = (embed[tgt] * np.float32(math.sqrt(D)) + _pos_enc()[None]).astype(f)
    boutr = np.broadcast_to(b_out, (128, ODIM)).copy()

    common = dict(
        identb=np.eye(128, dtype=bf), identr=np.eye(128, dtype=f),
        onesb=np.ones((1, 128), dtype=bf), onesr=np.ones((1, 128), dtype=f),
        wq=wq_f, wk=wk_f, wv=wv_f, wo=Wo.astype(bf), w1=w1_f, w2=W2.astype(bf),
        bq=bq_f, bk=bk_f, bv=bv_f, bo=bo.astype(bf), b1=b1_f, b2=b2.astype(bf),
        wenc=W_enc, benc=b_enc.astype(f).reshape(1, J),
        wdec=wdec_f, bdec=bdec_f,
        wout=W_out, boutr=boutr,
    )
    in_maps = []
    for c in range(NCORES):
        b, th = divmod(c, 2)
        mk = np.where(tgt_mask[b], f(0.0), f(-1e9)).astype(f)  # (U, U)
        m = dict(common)
        m["x0"] = x0_all[b]
        m["memT"] = np.ascontiguousarray(memory[b, th * TS : (th + 1) * TS, :].T)
        m["maskb"] = np.ascontiguousarray(np.tile(mk, (1, H)))
        in_maps.append(m)
    return in_maps, tgt_mask


def _gather(results):
    z = np.empty((B, T, U, ODIM), np.float32)
    for c in range(NCORES):
        b, th = divmod(c, 2)
        z[b, th * TS : (th + 1) * TS] = results[c]["out"].reshape(TS, U, ODIM)
    return z


def kernel(**inputs):
    from concourse.bass_utils import run_bass_kernel_spmd

    nc = _get_nc()
    in_maps, tgt_mask = _prep_maps(inputs)
    res = run_bass_kernel_spmd(nc, in_maps, list(range(NCORES)))
    return _gather(res.results), tgt_mask


def run_traced(inputs):
    """For test.py: returns (z, tgt_mask, BassKernelResults with timing)."""
    from concourse.bass_utils import run_bass_kernel_spmd

    nc = _get_nc()
    in_maps, tgt_mask = _prep_maps(inputs)
    res = run_bass_kernel_spmd(nc, in_maps, list(range(NCORES)), trace=True)
    return _gather(res.results), tgt_mask, res
